# revision 1
# baseline (speedup 1.0000x reference)
"""Local causal (sliding-window) attention on 8 Trainium2 NeuronCores.

Strategy: sequence-parallel. Each core owns 512 consecutive query tokens of
one batch element (cores 0-3 -> batch 0, cores 4-7 -> batch 1) plus a
128-token halo of preceding tokens whose k/v are recomputed locally, so no
inter-core communication is needed. The dataflow is fully transposed
(features on partitions, tokens on the free dim) so no on-device transposes
are required: the host pre-transposes each core's x-shard and transposes the
per-core output back while gathering. All matmuls run in float32r (fp22) at
full rate.

Attention per (supertile st of 256 queries, head h): the 256-query window
spans 3 key blocks r0/r1/r2 of 128 tokens. Masks are DVE-copied into PSUM
first and the QK matmuls accumulate onto them (start=False); the fully
masked quadrants of r0/r2 are never computed (half-width matmuls). exp runs
on ScalarE into SBUF p-tiles; AV accumulates v^T p with an extra ones
column in v producing the softmax denominators, which take a DMA round trip
through a 16-partition tile for one batched reciprocal, then a GpSimd
partition-broadcast feeds the normalizing multiply.
"""

import sys

sys.path.insert(0, "/opt/trn_rl_repo")
import numpy as np

B, S, D = 2, 2048, 1024
H, DH = 16, 64
WINDOW = 128
NCORES = 8
SLOC = 512            # queries per core
HALO = 128
TLOC = SLOC + HALO    # 640 local tokens (halo + queries)
NST = 2               # query supertiles of 256 per core
CPB = NCORES // B     # cores per batch element

_cached = {}


def _build():
    import concourse.bacc as bacc
    import concourse.mybir as mybir
    import concourse.tile as tile

    f32 = mybir.dt.float32
    f32r = mybir.dt.float32r
    bf16 = mybir.dt.bfloat16
    AF = mybir.ActivationFunctionType

    nc = bacc.Bacc(None)
    xT_d = nc.declare_dram_parameter("xT", [D, TLOC], f32, isOutput=False)
    wqkv_d = nc.declare_dram_parameter("w_qkv", [D, 3 * D], f32, isOutput=False)
    wout_d = nc.declare_dram_parameter("w_out", [D, D], f32, isOutput=False)
    mask_d = nc.declare_dram_parameter("mask", [NST, 128, 512], f32, isOutput=False)
    eye_d = nc.declare_dram_parameter("eye", [128, 128], f32, isOutput=False)
    ones_d = nc.declare_dram_parameter("ones", [128, H], f32, isOutput=False)
    out_d = nc.declare_dram_parameter("outT", [D, SLOC], f32, isOutput=True)

    with tile.TileContext(nc) as tc:
        with (
            tc.tile_pool(name="sb", bufs=1) as sb,
            tc.tile_pool(name="qkps", bufs=1, space="PSUM") as qkps,
            tc.tile_pool(name="scps", bufs=1, space="PSUM") as scps,
            tc.tile_pool(name="aops", bufs=1, space="PSUM") as aops,
        ):
            # ---- persistent SBUF tiles; DMA order: xt+ones, wq stream, masks, wout
            xt = [sb.tile([128, TLOC], f32r, tag=f"xt{k}", name=f"xt{k}") for k in range(8)]
            for k in range(8):
                nc.sync.dma_start(out=xt[k][:], in_=xT_d[k * 128:(k + 1) * 128, :].bitcast(f32r))
            ones_sb = sb.tile([128, H], f32r, tag="ones", name="ones_sb")
            nc.sync.dma_start(out=ones_sb[:], in_=ones_d[:].bitcast(f32r))

            qT = [sb.tile([128, SLOC], f32r, tag=f"qT{i}", name=f"qT{i}") for i in range(8)]
            kT = [sb.tile([128, TLOC], f32r, tag=f"kT{i}", name=f"kT{i}") for i in range(8)]
            vt = [sb.tile([128, 65 * H], f32r, tag=f"v{t}", name=f"v{t}") for t in range(5)]
            for t in range(5):
                v_ones = vt[t].rearrange("p (h c) -> p h c", c=65)[:, :, 64]
                nc.vector.tensor_copy(v_ones, ones_sb[:])
            att = [sb.tile([128, SLOC], f32r, tag=f"at{t}", name=f"at{t}") for t in range(8)]

            def wq_dma(cb):
                tiles = []
                for k in range(8):
                    wqk = sb.tile([128, 512], f32r, tag="wq", bufs=16, name=f"wq{cb}_{k}")
                    nc.sync.dma_start(
                        out=wqk[:],
                        in_=wqkv_d[k * 128:(k + 1) * 128, cb * 512:(cb + 1) * 512].bitcast(f32r),
                    )
                    tiles.append(wqk)
                return tiles

            # ---- phase 1: qkv projection ----
            for cb in range(2):            # q columns; queries only
                wq = wq_dma(cb)
                for m in range(4):
                    ps = qkps.tile([128, 512], f32, tag="qk", bufs=2, name=f"psq{cb}_{m}")
                    for k in range(8):
                        nc.tensor.matmul(
                            ps[:], wq[k][:, m * 128:(m + 1) * 128], xt[k][:, HALO:TLOC],
                            start=(k == 0), stop=(k == 7),
                        )
                    nc.scalar.copy(qT[cb * 4 + m][:], ps[:])
            for cb in range(2, 4):         # k columns; all 640 tokens
                wq = wq_dma(cb)
                for m in range(4):
                    for n in range(2):
                        ps = qkps.tile([128, 320], f32, tag="qk", bufs=2, name=f"psk{cb}_{m}_{n}")
                        for k in range(8):
                            nc.tensor.matmul(
                                ps[:], wq[k][:, m * 128:(m + 1) * 128],
                                xt[k][:, n * 320:(n + 1) * 320],
                                start=(k == 0), stop=(k == 7),
                            )
                        nc.scalar.copy(kT[(cb - 2) * 4 + m][:, n * 320:(n + 1) * 320], ps[:])
            # v columns: token-tile-major across both column halves so vt[t]
            # completes in jb order for the attention pipeline
            msk = [sb.tile([128, 512], f32r, tag=f"mk{i}", name=f"mk{i}") for i in range(NST)]
            eye_sb = sb.tile([128, 128], f32r, tag="eye", name="eye_sb")
            nc.sync.dma_start(out=eye_sb[:], in_=eye_d[:].bitcast(f32r))
            for st in range(NST):
                nc.sync.dma_start(out=msk[st][:], in_=mask_d[st].bitcast(f32r))
            wq4 = wq_dma(4)
            wq5 = wq_dma(5)
            for t in range(5):
                for half, wq in ((0, wq4), (1, wq5)):
                    ps = qkps.tile([128, 512], f32, tag="qk", bufs=2, name=f"psv{t}_{half}")
                    for k in range(8):
                        nc.tensor.matmul(
                            ps[:], xt[k][:, t * 128:(t + 1) * 128], wq[k][:, :],
                            start=(k == 0), stop=(k == 7),
                        )
                    h0 = half * 8
                    dst = vt[t].rearrange("p (h c) -> p h c", c=65)[:, h0:h0 + 8, 0:64]
                    src = ps[:].rearrange("p (h c) -> p h c", c=64)
                    nc.scalar.copy(dst, src)
            # w_out reuses the streamed-weight slots: half A = cols 0:512 of row
            # block k (proj m 0..3), half B = cols 512:1024 (m 4..7)
            woA = []
            woB = []
            for k in range(8):
                wa = sb.tile([128, 512], f32r, tag="wq", bufs=16, name=f"woA{k}")
                nc.sync.dma_start(out=wa[:], in_=wout_d[k * 128:(k + 1) * 128, 0:512].bitcast(f32r))
                woA.append(wa)
            for k in range(8):
                wb = sb.tile([128, 512], f32r, tag="wq", bufs=16, name=f"woB{k}")
                nc.sync.dma_start(out=wb[:], in_=wout_d[k * 128:(k + 1) * 128, 512:1024].bitcast(f32r))
                woB.append(wb)

            # ---- phase 2+3: attention and output projection ----
            DEPTH = 3
            scat = sb.tile([1, H * 256], f32, tag="scat", name="scat")
            rcat = sb.tile([1, H * 256], f32, tag="rcat", name="rcat")
            for st in range(NST):
                q0 = st * 256
                pend = {}

                def emit_qk(h, st=st, q0=q0, pend=pend):
                    t, poff = h // 2, (h % 2) * 64
                    jb = st * 2
                    sc = scps.tile([128, 512], f32, tag="sc", bufs=4, name=f"sc_{st}_{h}")
                    nc.tensor.matmul(
                        sc[:], eye_sb[:], msk[st][:],
                        start=True, stop=False, skip_group_check=True,
                    )
                    nc.tensor.matmul(
                        sc[:, 0:128],
                        kT[t][poff:poff + 64, jb * 128:(jb + 1) * 128],
                        qT[t][poff:poff + 64, q0:q0 + 128],
                        start=False, stop=False, skip_group_check=True,
                    )
                    nc.tensor.matmul(
                        sc[:, 128:256],
                        kT[t][poff:poff + 64, (jb + 2) * 128:(jb + 3) * 128],
                        qT[t][poff:poff + 64, q0 + 128:q0 + 256],
                        start=False, stop=False, skip_group_check=True,
                    )
                    nc.tensor.matmul(
                        sc[:, 256:512],
                        kT[t][poff:poff + 64, (jb + 1) * 128:(jb + 2) * 128],
                        qT[t][poff:poff + 64, q0:q0 + 256],
                        start=False, stop=True, skip_group_check=True,
                    )
                    p = sb.tile([128, 512], f32r, tag="pp", bufs=DEPTH + 3, name=f"p_{st}_{h}")
                    nc.scalar.activation(p[:], sc[:], AF.Exp, scale=0.125)
                    pend[h] = p

                def emit_av(h, st=st, q0=q0, pend=pend):
                    t, poff = h // 2, (h % 2) * 64
                    jb = st * 2
                    p = pend.pop(h)
                    av = aops.tile([65, 256], f32, tag="ao", bufs=2, name=f"av{st}_{h}")
                    nc.tensor.matmul(
                        av[:], vt[jb + 1][:, h * 65:h * 65 + 65], p[:, 256:512],
                        start=True, stop=False, skip_group_check=True,
                    )
                    nc.tensor.matmul(
                        av[:, 0:128], vt[jb][:, h * 65:h * 65 + 65], p[:, 0:128],
                        start=False, stop=False, skip_group_check=True,
                    )
                    nc.tensor.matmul(
                        av[:, 128:256], vt[jb + 2][:, h * 65:h * 65 + 65], p[:, 128:256],
                        start=False, stop=True, skip_group_check=True,
                    )
                    nc.scalar.copy(scat[0:1, h * 256:(h + 1) * 256], av[64:65, :])
                    nc.vector.tensor_copy(att[t][poff:poff + 64, q0:q0 + 256], av[0:64, :])

                for step in range(H + DEPTH):
                    if step < H:
                        emit_qk(step)
                    if step >= DEPTH:
                        emit_av(step - DEPTH)

                # batched softmax denominators
                s16 = sb.tile([16, 256], f32, tag="s16", bufs=2, name=f"s16_{st}")
                for h in range(H):
                    nc.sync.dma_start(out=s16[h:h + 1, :], in_=scat[0:1, h * 256:(h + 1) * 256])
                r16 = sb.tile([16, 256], f32, tag="r16", bufs=2, name=f"r16_{st}")
                nc.vector.reciprocal(r16[:], s16[:])
                for h in range(H):
                    nc.sync.dma_start(out=rcat[0:1, h * 256:(h + 1) * 256], in_=r16[h:h + 1, :])
                for h in range(H):
                    t, poff = h // 2, (h % 2) * 64
                    rb = sb.tile([128, 256], f32, tag="rb", bufs=4, name=f"rb{st}_{h}")
                    nc.gpsimd.partition_broadcast(rb[:], rcat[0:1, h * 256:(h + 1) * 256])
                    asl = att[t][poff:poff + 64, q0:q0 + 256]
                    nc.vector.tensor_mul(asl, asl, rb[poff:poff + 64, :])
                # output projection for this supertile
                for m in range(8):
                    wo = woA if m < 4 else woB
                    mc = (m % 4) * 128
                    po = aops.tile([128, 256], f32, tag="ao", bufs=2, name=f"po{st}_{m}")
                    for k in range(8):
                        nc.tensor.matmul(
                            po[:], wo[k][:, mc:mc + 128], att[k][:, q0:q0 + 256],
                            start=(k == 0), stop=(k == 7),
                        )
                    ot = sb.tile([128, 256], f32, tag="ot", bufs=4, name=f"ot{st}_{m}")
                    nc.scalar.copy(ot[:], po[:])
                    nc.sync.dma_start(
                        out=out_d[m * 128:(m + 1) * 128, q0:q0 + 256], in_=ot[:],
                    )

    nc.finalize()
    return nc


def _get_nc():
    if "nc" not in _cached:
        _cached["nc"] = _build()
    return _cached["nc"]


def _core_inputs(x, w_qkv, w_out):
    in_maps = []
    for c in range(NCORES):
        b, qs = c // CPB, (c % CPB) * SLOC
        xs = np.zeros((TLOC, D), dtype=np.float32)
        lo = max(0, qs - HALO)
        xs[HALO - (qs - lo):] = x[b, lo:qs + SLOC]
        # masks: additive bias on raw scores (exp applies the 0.125 scale).
        # mask[st][0] packs [r0 x queries 0:128 | r2 x queries 128:256];
        # mask[st][1] is r1 (middle key block) for all 256 queries.
        i = np.arange(256)[None, None, None, :]
        j = np.arange(128)[None, None, :, None]
        st = np.arange(NST)[:, None, None, None]
        r = np.arange(3)[None, :, None, None]
        qg = qs + st * 256 + i
        kg = qs + st * 256 - HALO + r * 128 + j
        allowed = (kg <= qg) & (kg > qg - WINDOW) & (kg >= 0)
        m3 = np.where(allowed, 0.0, -8e30).astype(np.float32)
        mask = np.empty((NST, 128, 512), dtype=np.float32)
        mask[:, :, 0:128] = m3[:, 0, :, 0:128]
        mask[:, :, 128:256] = m3[:, 2, :, 128:256]
        mask[:, :, 256:512] = m3[:, 1]
        in_maps.append(
            {
                "xT": np.ascontiguousarray(xs.T),
                "w_qkv": np.ascontiguousarray(w_qkv, dtype=np.float32),
                "w_out": np.ascontiguousarray(w_out, dtype=np.float32),
                "mask": mask,
                "ones": np.ones((128, H), dtype=np.float32),
                "eye": np.eye(128, dtype=np.float32),
            }
        )
    return in_maps


def kernel(x, w_qkv, w_out, _trace=False, _trace_kwargs=None):
    from concourse.bass_utils import run_bass_kernel_spmd

    x = np.asarray(x, dtype=np.float32)
    w_qkv = np.asarray(w_qkv, dtype=np.float32)
    w_out = np.asarray(w_out, dtype=np.float32)
    nc = _get_nc()
    in_maps = _core_inputs(x, w_qkv, w_out)
    res = run_bass_kernel_spmd(
        nc, in_maps, list(range(NCORES)), trace=_trace, **(_trace_kwargs or {})
    )
    out = np.concatenate(
        [res.results[c]["outT"].T for c in range(NCORES)], axis=0
    ).reshape(B, S, D)
    if _trace:
        return out, res
    return out



# revision 3
# speedup vs baseline: 1.3061x; 1.3061x over previous
"""Local causal (sliding-window) attention on 8 Trainium2 NeuronCores.

Sequence-parallel: each core owns 512 consecutive query tokens of one batch
element (cores 0-3 -> batch 0, 4-7 -> batch 1) plus a 128-token halo whose
k/v are recomputed locally, so no inter-core communication is needed.

All matmuls run in bf16 (full PE rate at any moving size, half the HBM
bytes of fp32); PSUM accumulates fp32. Every input is host-prebaked into
the exact SBUF layout so each tensor arrives in one fully contiguous DMA.

Attention per (supertile st of 256 queries, head h): the 256-query window
spans 3 key blocks r0/r1/r2. Scores are computed unmasked, exp'd on ScalarE
(scale=1/8) into bf16, then multiplied by a binary {0,1} mask on the Vector
engine (keeps the PE free of mask matmuls). AV accumulates v^T p with an
extra ones column producing softmax denominators on PSUM row 64; per
supertile the 16 denominator rows take one batched SBUF->SBUF DMA into a
[16,256] tile, one reciprocal_approx_fast, and one DMA back, then GpSimd
partition-broadcasts feed the normalizing multiplies.

PE gaps are filled by emission interleaving: the last two v-projection
token-tiles overlap st0's attention, and st0's output projection overlaps
st1's attention, hiding the exp/denominator latency.
"""

import sys

sys.path.insert(0, "/opt/trn_rl_repo")
import numpy as np
import ml_dtypes

BF16 = ml_dtypes.bfloat16

B, S, D = 2, 2048, 1024
H, DH = 16, 64
WINDOW = 128
NCORES = 8
SLOC = 512            # queries per core
HALO = 128
TLOC = SLOC + HALO    # 640 local tokens (halo + queries)
NST = 2               # query supertiles of 256 per core
CPB = NCORES // B     # cores per batch element

_cached = {}


def _build():
    import concourse.bacc as bacc
    import concourse.mybir as mybir
    import concourse.tile as tile

    f32 = mybir.dt.float32
    bf16 = mybir.dt.bfloat16
    AF = mybir.ActivationFunctionType

    nc = bacc.Bacc(None)
    xt_d = nc.declare_dram_parameter("xt", [128, 8 * TLOC], bf16, isOutput=False)
    wq_d = nc.declare_dram_parameter("wq", [128, 6 * 4096], bf16, isOutput=False)
    wo_d = nc.declare_dram_parameter("wo", [128, 8192], bf16, isOutput=False)
    cst_d = nc.declare_dram_parameter("cst", [128, NST * 512 + 16], bf16, isOutput=False)
    out_d = nc.declare_dram_parameter("out", [128, NST * 2048], bf16, isOutput=True)

    with tile.TileContext(nc) as tc:
        with (
            tc.tile_pool(name="sb", bufs=1) as sb,
            tc.tile_pool(name="pjps", bufs=1, space="PSUM") as pjps,
            tc.tile_pool(name="scps", bufs=1, space="PSUM") as scps,
            tc.tile_pool(name="avps", bufs=1, space="PSUM") as avps,
        ):
            # ---- persistent SBUF tiles; DMA order: xt, cst, wq stream, wo
            xt = sb.tile([128, 8 * TLOC], bf16, tag="xt", name="xt")
            nc.sync.dma_start(out=xt[:], in_=xt_d[:])
            cst = sb.tile([128, NST * 512 + 16], bf16, tag="cst", name="cst")
            nc.sync.dma_start(out=cst[:], in_=cst_d[:])
            wq = []
            for cb in range(6):
                w = sb.tile([128, 4096], bf16, tag="wq", bufs=3, name=f"wq{cb}")
                nc.sync.dma_start(out=w[:], in_=wq_d[:, cb * 4096:(cb + 1) * 4096])
                wq.append(w)
            woA = sb.tile([128, 4096], bf16, tag="woA", name="woA")
            nc.sync.dma_start(out=woA[:], in_=wo_d[:, 0:4096])
            woB = sb.tile([128, 4096], bf16, tag="woB", name="woB")
            nc.sync.dma_start(out=woB[:], in_=wo_d[:, 4096:8192])

            msk = [cst[:, 0:512], cst[:, 512:1024]]
            ones_sb = cst[:, 1024:1040]

            qT = [sb.tile([128, SLOC], bf16, tag=f"qT{i}", name=f"qT{i}") for i in range(8)]
            kT = [sb.tile([128, TLOC], bf16, tag=f"kT{i}", name=f"kT{i}") for i in range(8)]
            vt = [sb.tile([128, 65 * H], bf16, tag=f"v{t}", name=f"v{t}") for t in range(5)]
            att = [sb.tile([128, SLOC], bf16, tag=f"at{t}", name=f"at{t}") for t in range(8)]
            ot = [sb.tile([128, 2048], bf16, tag=f"ot{st}", name=f"ot{st}") for st in range(NST)]
            for t in range(5):
                v_ones = vt[t].rearrange("p (h c) -> p h c", c=65)[:, :, 64]
                nc.vector.tensor_copy(v_ones, ones_sb[:])

            # ---- phase 1: qkv projection ----
            for cb in range(2):            # q columns; queries only
                for m in range(4):
                    ps = pjps.tile([128, 512], f32, tag="qk", bufs=2, name=f"psq{cb}_{m}")
                    for k in range(8):
                        nc.tensor.matmul(
                            ps[:],
                            wq[cb][:, k * 512 + m * 128:k * 512 + (m + 1) * 128],
                            xt[:, k * TLOC + HALO:(k + 1) * TLOC],
                            start=(k == 0), stop=(k == 7),
                        )
                    nc.scalar.copy(qT[cb * 4 + m][:], ps[:])
            for cb in range(2, 4):         # k columns; all 640 tokens
                for m in range(4):
                    for n in range(2):
                        ps = pjps.tile([128, 320], f32, tag="qk", bufs=2, name=f"psk{cb}_{m}_{n}")
                        for k in range(8):
                            nc.tensor.matmul(
                                ps[:],
                                wq[cb][:, k * 512 + m * 128:k * 512 + (m + 1) * 128],
                                xt[:, k * TLOC + n * 320:k * TLOC + (n + 1) * 320],
                                start=(k == 0), stop=(k == 7),
                            )
                        nc.scalar.copy(kT[(cb - 2) * 4 + m][:, n * 320:(n + 1) * 320], ps[:])

            # v columns, token-tile-major. t=0..2 emitted here; t=3,4 become
            # fill work interleaved into st0's attention steps.
            def v_group_thunks(t, half):
                ps = pjps.tile([128, 512], f32, tag="qk", bufs=2, name=f"psv{t}_{half}")
                thunks = []
                for k in range(8):
                    def mm(k=k, ps=ps, t=t, half=half):
                        nc.tensor.matmul(
                            ps[:],
                            xt[:, k * TLOC + t * 128:k * TLOC + (t + 1) * 128],
                            wq[4 + half][:, k * 512:(k + 1) * 512],
                            start=(k == 0), stop=(k == 7),
                        )
                    thunks.append(mm)

                def cp(ps=ps, t=t, half=half):
                    h0 = half * 8
                    dst = vt[t].rearrange("p (h c) -> p h c", c=65)[:, h0:h0 + 8, 0:64]
                    src = ps[:].rearrange("p (h c) -> p h c", c=64)
                    nc.scalar.copy(dst, src)
                thunks.append(cp)
                return thunks

            for t in range(3):
                for half in range(2):
                    for th in v_group_thunks(t, half):
                        th()
            fill_st0 = []
            for t in range(3, 5):
                for half in range(2):
                    fill_st0.extend(v_group_thunks(t, half))

            # out-projection thunks for one supertile-block of 4 m's
            def po_thunks(st, m0, m1):
                thunks = []
                q0 = st * 256
                for m in range(m0, m1):
                    po = pjps.tile([128, 256], f32, tag="qk", bufs=2, name=f"po{st}_{m}")
                    for k in range(8):
                        def mm(k=k, po=po, m=m, q0=q0):
                            wo = woA if k < 4 else woB
                            kk = k % 4
                            nc.tensor.matmul(
                                po[:],
                                wo[:, kk * 1024 + m * 128:kk * 1024 + (m + 1) * 128],
                                att[k][:, q0:q0 + 256],
                                start=(k == 0), stop=(k == 7),
                            )
                        thunks.append(mm)

                    def cp(po=po, m=m, st=st):
                        nc.scalar.copy(ot[st][:, m * 256:(m + 1) * 256], po[:])
                    thunks.append(cp)
                return thunks

            # ---- phase 2+3: attention, denominators, output projection ----
            LAG = 4   # heads between emit_qk(h) and the av consuming p(h)

            def attn_st(st, fill, fill_start, per_step):
                q0 = st * 256
                jb = st * 2
                pend = {}
                scat = sb.tile([1, 4096], f32, tag="scat", bufs=2, name=f"scat{st}")

                def emit_qk(h):
                    t, poff = h // 2, (h % 2) * 64
                    sc = scps.tile([128, 512], f32, tag="sc", bufs=4, name=f"sc{st}_{h}")
                    nc.tensor.matmul(
                        sc[:, 256:512],
                        kT[t][poff:poff + 64, (jb + 1) * 128:(jb + 2) * 128],
                        qT[t][poff:poff + 64, q0:q0 + 256],
                        start=True, stop=False, skip_group_check=True,
                    )
                    nc.tensor.matmul(
                        sc[:, 0:128],
                        kT[t][poff:poff + 64, jb * 128:(jb + 1) * 128],
                        qT[t][poff:poff + 64, q0:q0 + 128],
                        start=True, stop=False, skip_group_check=True,
                    )
                    nc.tensor.matmul(
                        sc[:, 128:256],
                        kT[t][poff:poff + 64, (jb + 2) * 128:(jb + 3) * 128],
                        qT[t][poff:poff + 64, q0 + 128:q0 + 256],
                        start=True, stop=True, skip_group_check=True,
                    )
                    p = sb.tile([128, 512], bf16, tag="pp", bufs=8, name=f"p{st}_{h}")
                    nc.scalar.activation(p[:], sc[:], AF.Exp, scale=0.125)
                    nc.vector.tensor_mul(p[:], p[:], msk[st])
                    pend[h] = p

                def emit_av_pair(j):
                    t = j
                    p0, p1 = pend.pop(2 * j), pend.pop(2 * j + 1)
                    av = avps.tile([65, 512], f32, tag="av", bufs=2, name=f"av{st}_{j}")
                    for half, p in ((0, p0), (1, p1)):
                        c0 = half * 256
                        h = 2 * j + half
                        nc.tensor.matmul(
                            av[:, c0:c0 + 256], vt[jb + 1][:, h * 65:h * 65 + 65],
                            p[:, 256:512],
                            start=True, stop=False, skip_group_check=True,
                        )
                        nc.tensor.matmul(
                            av[:, c0:c0 + 128], vt[jb][:, h * 65:h * 65 + 65],
                            p[:, 0:128],
                            start=False, stop=False, skip_group_check=True,
                        )
                        nc.tensor.matmul(
                            av[:, c0 + 128:c0 + 256], vt[jb + 2][:, h * 65:h * 65 + 65],
                            p[:, 128:256],
                            start=False, stop=True, skip_group_check=True,
                        )
                    nc.scalar.copy(scat[0:1, j * 512:(j + 1) * 512], av[64:65, :])
                    nc.vector.tensor_copy(att[t][0:64, q0:q0 + 256], av[0:64, 0:256])
                    nc.vector.tensor_copy(att[t][64:128, q0:q0 + 256], av[0:64, 256:512])

                fi = 0
                for h in range(H + LAG):
                    if h < H:
                        emit_qk(h)
                    if h >= fill_start:
                        for _ in range(per_step):
                            if fi < len(fill):
                                fill[fi]()
                                fi += 1
                    if h >= LAG and (h - LAG) % 2 == 1:
                        emit_av_pair((h - LAG) // 2)
                while fi < len(fill):
                    fill[fi]()
                    fi += 1

                # batched softmax denominators: one DMA in, one reciprocal,
                # one DMA out, then per-pair broadcasts + normalize muls
                s16 = sb.tile([16, 256], f32, tag="s16", bufs=2, name=f"s16_{st}")
                nc.sync.dma_start(out=s16[:], in_=scat[0:1, :])
                r16 = sb.tile([16, 256], f32, tag="r16", bufs=2, name=f"r16_{st}")
                nc.vector.reciprocal_approx_fast(out=r16[:], in_=s16[:])
                r16b = sb.tile([16, 256], bf16, tag="r16b", bufs=2, name=f"r16b_{st}")
                nc.vector.tensor_copy(r16b[:], r16[:])
                rcat = sb.tile([1, 4096], bf16, tag="rcat", bufs=2, name=f"rcat{st}")
                nc.sync.dma_start(out=rcat[0:1, :], in_=r16b[:])
                for h in range(H):
                    t, poff = h // 2, (h % 2) * 64
                    rb = sb.tile([128, 256], bf16, tag="rb", bufs=4, name=f"rb{st}_{h}")
                    nc.gpsimd.partition_broadcast(
                        rb[:], rcat[0:1, h * 256:(h + 1) * 256]
                    )
                    asl = att[t][poff:poff + 64, q0:q0 + 256]
                    nc.vector.tensor_mul(asl, asl, rb[poff:poff + 64, :])

            attn_st(0, fill_st0, fill_start=0, per_step=3)
            attn_st(1, po_thunks(0, 0, 4), fill_start=6, per_step=4)
            for th in po_thunks(0, 4, 8):
                th()
            nc.sync.dma_start(out=out_d[:, 0:2048], in_=ot[0][:])
            for th in po_thunks(1, 0, 8):
                th()
            nc.sync.dma_start(out=out_d[:, 2048:4096], in_=ot[1][:])

    nc.finalize()
    return nc


def _get_nc():
    if "nc" not in _cached:
        _cached["nc"] = _build()
    return _cached["nc"]


def _core_inputs(x, w_qkv, w_out):
    # shared, host-prebaked weight layouts (bf16, exact SBUF layout)
    wq_h = np.ascontiguousarray(
        w_qkv.reshape(8, 128, 6, 512).transpose(1, 2, 0, 3).reshape(128, 6 * 4096)
    ).astype(BF16)
    wo_h = np.ascontiguousarray(
        w_out.reshape(8, 128, 1024).transpose(1, 0, 2).reshape(128, 8192)
    ).astype(BF16)

    in_maps = []
    for c in range(NCORES):
        b, qs = c // CPB, (c % CPB) * SLOC
        xs = np.zeros((TLOC, D), dtype=np.float32)
        lo = max(0, qs - HALO)
        xs[HALO - (qs - lo):] = x[b, lo:qs + SLOC]
        xt_h = np.ascontiguousarray(
            xs.T.reshape(8, 128, TLOC).transpose(1, 0, 2).reshape(128, 8 * TLOC)
        ).astype(BF16)

        # binary {0,1} masks multiplying exp'd scores.
        # mask[st][:, 0:128] = r0 x queries 0:128; [:, 128:256] = r2 x
        # queries 128:256; [:, 256:512] = r1 x all 256 queries.
        i = np.arange(256)[None, None, None, :]
        j = np.arange(128)[None, None, :, None]
        st = np.arange(NST)[:, None, None, None]
        r = np.arange(3)[None, :, None, None]
        qg = qs + st * 256 + i
        kg = qs + st * 256 - HALO + r * 128 + j
        allowed = (kg <= qg) & (kg > qg - WINDOW) & (kg >= 0)
        m3 = allowed.astype(np.float32)
        mask = np.empty((NST, 128, 512), dtype=np.float32)
        mask[:, :, 0:128] = m3[:, 0, :, 0:128]
        mask[:, :, 128:256] = m3[:, 2, :, 128:256]
        mask[:, :, 256:512] = m3[:, 1]
        cst_h = np.empty((128, NST * 512 + 16), dtype=BF16)
        cst_h[:, 0:512] = mask[0]
        cst_h[:, 512:1024] = mask[1]
        cst_h[:, 1024:1040] = 1.0

        in_maps.append(
            {"xt": xt_h, "wq": wq_h, "wo": wo_h, "cst": cst_h}
        )
    return in_maps


def kernel(x, w_qkv, w_out, _trace=False, _trace_kwargs=None):
    from concourse.bass_utils import run_bass_kernel_spmd

    x = np.asarray(x, dtype=np.float32)
    w_qkv = np.asarray(w_qkv, dtype=np.float32)
    w_out = np.asarray(w_out, dtype=np.float32)
    nc = _get_nc()
    in_maps = _core_inputs(x, w_qkv, w_out)
    res = run_bass_kernel_spmd(
        nc, in_maps, list(range(NCORES)), trace=_trace, **(_trace_kwargs or {})
    )
    out = np.empty((B, S, D), dtype=np.float32)
    for c in range(NCORES):
        b, qs = c // CPB, (c % CPB) * SLOC
        o = np.asarray(res.results[c]["out"], dtype=np.float32)
        # o[p, st*2048 + m*256 + c2] = out[b, qs + st*256 + c2, m*128 + p]
        out[b, qs:qs + SLOC] = (
            o.reshape(128, NST, 8, 256).transpose(1, 3, 2, 0).reshape(SLOC, D)
        )
    if _trace:
        return out, res
    return out


# revision 10
# speedup vs baseline: 1.3727x; 1.0510x over previous
"""Local causal (sliding-window) attention on 8 Trainium2 NeuronCores.

Sequence-parallel: each core owns 512 consecutive query tokens of one batch
element (cores 0-3 -> batch 0, 4-7 -> batch 1) plus a 128-token halo whose
k/v are recomputed locally, so no inter-core communication is needed.

All matmuls run in bf16 (full PE rate at any moving size, half the HBM
bytes of fp32); PSUM accumulates fp32. Every input is host-prebaked into
the exact SBUF layout so each tensor arrives in a few fully contiguous
DMAs, issued in consumption order with dependency-gated staggering so the
round-robin DMA engine doesn't dilute early transfers with late weights.

The emission schedule software-pipelines engines: st0's attention pairs are
interleaved with the remaining projection groups (q-cb1, k-cb3, v halves),
st0's output projection fills st1's attention gaps, and denominators are
processed in sub-batches (batched SBUF->SBUF DMA to a [2n,256] tile,
reciprocal_approx_fast, casting DMA back, GpSimd partition-broadcasts) so
their serial chain hides under PE work.
"""

import sys

sys.path.insert(0, "/opt/trn_rl_repo")
import numpy as np
import ml_dtypes

BF16 = ml_dtypes.bfloat16

B, S, D = 2, 2048, 1024
H, DH = 16, 64
WINDOW = 128
NCORES = 8
SLOC = 512
HALO = 128
TLOC = SLOC + HALO
NST = 2
CPB = NCORES // B

_cached = {}


def _build():
    import concourse.bacc as bacc
    import concourse.mybir as mybir
    import concourse.tile as tile

    f32 = mybir.dt.float32
    bf16 = mybir.dt.bfloat16
    AF = mybir.ActivationFunctionType

    nc = bacc.Bacc(None)
    xt_d = nc.declare_dram_parameter("xt", [128, 8 * TLOC], bf16, isOutput=False)
    wq_d = nc.declare_dram_parameter("wq", [128, 6 * 4096], bf16, isOutput=False)
    wo_d = nc.declare_dram_parameter("wo", [128, 8192], bf16, isOutput=False)
    cst_d = nc.declare_dram_parameter("cst", [128, NST * 512 + 16], bf16, isOutput=False)
    out_d = nc.declare_dram_parameter("out", [128, NST * 2048], bf16, isOutput=True)

    with tile.TileContext(nc) as tc:
        with (
            tc.tile_pool(name="sb", bufs=1) as sb,
            tc.tile_pool(name="pjps", bufs=1, space="PSUM") as pjps,
            tc.tile_pool(name="scps", bufs=1, space="PSUM") as scps,
            tc.tile_pool(name="avps", bufs=1, space="PSUM") as avps,
        ):
            # ---- head DMAs, consumption-ordered. wq0 is m-major (4 chunks
            # so the first q-group gates on only 0.25MB of weights); xt in 2
            # halves so the first k-accumulation starts after half the x.
            wq0m = [sb.tile([128, 1024], bf16, tag=f"wq0m{m}", name=f"wq0m{m}")
                    for m in range(4)]
            nc.sync.dma_start(out=wq0m[0][:], in_=wq_d[:, 0:1024])
            xta = sb.tile([128, 4 * TLOC], bf16, tag="xta", name="xta")
            nc.sync.dma_start(out=xta[:], in_=xt_d[:, 0:4 * TLOC])
            xtb = sb.tile([128, 4 * TLOC], bf16, tag="xtb", name="xtb")
            nc.sync.dma_start(out=xtb[:], in_=xt_d[:, 4 * TLOC:8 * TLOC])

            def xt_sl(k, c0, c1):
                t = xta if k < 4 else xtb
                kk = k % 4
                return t[:, kk * TLOC + c0:kk * TLOC + c1]

            for m in range(1, 4):
                nc.sync.dma_start(
                    out=wq0m[m][:], in_=wq_d[:, m * 1024:(m + 1) * 1024]
                )
            cst = sb.tile([128, NST * 512 + 16], bf16, tag="cst", name="cst")
            nc.sync.dma_start(out=cst[:], in_=cst_d[:])
            wq2 = sb.tile([128, 4096], bf16, tag="wq2", name="wq2")
            nc.sync.dma_start(out=wq2[:], in_=wq_d[:, 2 * 4096:3 * 4096])
            # later weight tiles are declared now but DMA'd behind a tiny
            # WAW "gate" write that keys each transfer to pipeline progress,
            # so early transfers keep full DMA bandwidth.
            wq4 = sb.tile([128, 4096], bf16, tag="wq4", name="wq4")
            wq5 = sb.tile([128, 4096], bf16, tag="wq5", name="wq5")
            wq1 = sb.tile([128, 4096], bf16, tag="wq1", name="wq1")
            wq3 = sb.tile([128, 4096], bf16, tag="wq3", name="wq3")
            woA = sb.tile([128, 4096], bf16, tag="woA", name="woA")
            woB = sb.tile([128, 4096], bf16, tag="woB", name="woB")

            def gated_dma(dst, col0, key_ap):
                nc.vector.tensor_copy(dst[0:1, 0:8], key_ap)
                nc.sync.dma_start(out=dst[:], in_=wq_d[:, col0:col0 + 4096]
                                  if col0 < 6 * 4096 else wo_d[:, col0 - 6 * 4096:col0 - 6 * 4096 + 4096])

            msk = [cst[:, 0:512], cst[:, 512:1024]]
            ones_sb = cst[:, 1024:1040]

            qT = [sb.tile([128, SLOC], bf16, tag=f"qT{i}", name=f"qT{i}") for i in range(8)]
            kT = [sb.tile([128, TLOC], bf16, tag=f"kT{i}", name=f"kT{i}") for i in range(8)]
            vt = [sb.tile([128, 65 * H], bf16, tag=f"v{t}", name=f"v{t}") for t in range(5)]
            att = [[sb.tile([128, 256], bf16, tag=f"at{st}_{t}", name=f"at{st}_{t}")
                    for t in range(8)] for st in range(NST)]
            ot = [sb.tile([128, 2048], bf16, tag=f"ot{st}", name=f"ot{st}") for st in range(NST)]

            # ---- projection group emitters (thunk lists of single ops) ----
            def q_group(cb, m):
                ps = pjps.tile([128, 512], f32, tag="qk", bufs=2, name=f"psq{cb}_{m}")
                th = []
                for k in range(8):
                    def mm(k=k, ps=ps, cb=cb, m=m):
                        if cb == 0:
                            lhs = wq0m[m][:, k * 128:(k + 1) * 128]
                        else:
                            lhs = wq1[:, k * 512 + m * 128:k * 512 + (m + 1) * 128]
                        nc.tensor.matmul(
                            ps[:], lhs, xt_sl(k, HALO, TLOC),
                            start=(k == 0), stop=(k == 7),
                        )
                    th.append(mm)
                th.append(lambda ps=ps, cb=cb, m=m: nc.scalar.copy(qT[cb * 4 + m][:], ps[:]))
                return th

            def k_group(cb, m, n):
                w = wq2 if cb == 2 else wq3
                ps = pjps.tile([128, 320], f32, tag="qk", bufs=2, name=f"psk{cb}_{m}_{n}")
                th = []
                for k in range(8):
                    def mm(k=k, ps=ps, w=w, m=m, n=n):
                        nc.tensor.matmul(
                            ps[:], w[:, k * 512 + m * 128:k * 512 + (m + 1) * 128],
                            xt_sl(k, n * 320, (n + 1) * 320),
                            start=(k == 0), stop=(k == 7),
                        )
                    th.append(mm)
                th.append(lambda ps=ps, cb=cb, m=m, n=n: nc.scalar.copy(
                    kT[(cb - 2) * 4 + m][:, n * 320:(n + 1) * 320], ps[:]))
                return th

            def v_group(t, half):
                w = wq4 if half == 0 else wq5
                ps = pjps.tile([128, 512], f32, tag="qk", bufs=2, name=f"psv{t}_{half}")
                th = []
                for k in range(8):
                    def mm(k=k, ps=ps, w=w, t=t):
                        nc.tensor.matmul(
                            ps[:], xt_sl(k, t * 128, (t + 1) * 128),
                            w[:, k * 512:(k + 1) * 512],
                            start=(k == 0), stop=(k == 7),
                        )
                    th.append(mm)

                def cp(ps=ps, t=t, half=half):
                    h0 = half * 8
                    dst = vt[t].rearrange("p (h c) -> p h c", c=65)[:, h0:h0 + 8, 0:64]
                    nc.scalar.copy(dst, ps[:].rearrange("p (h c) -> p h c", c=64))
                th.append(cp)
                return th

            def po2_group(st, g):
                q0 = st * 256
                po = pjps.tile([128, 512], f32, tag="qk", bufs=2, name=f"po{st}_{g}")
                th = []
                for half in range(2):
                    m = 2 * g + half
                    c0 = half * 256
                    for k in range(8):
                        def mm(k=k, po=po, m=m, c0=c0, st=st, q0=q0):
                            wo = woA if k < 4 else woB
                            kk = k % 4
                            nc.tensor.matmul(
                                po[:, c0:c0 + 256],
                                wo[:, kk * 1024 + m * 128:kk * 1024 + (m + 1) * 128],
                                att[st][k][:, :],
                                start=(k == 0), stop=(k == 7),
                                skip_group_check=True,
                            )
                        th.append(mm)
                th.append(lambda po=po, st=st, g=g: nc.scalar.copy(
                    ot[st][:, g * 512:(g + 1) * 512], po[:]))
                return th

            # ---- attention emitters ----
            pend = {}

            def emit_qk(st, h):
                q0 = st * 256
                jb = st * 2
                t, poff = h // 2, (h % 2) * 64
                sc = scps.tile([128, 512], f32, tag="sc", bufs=4, name=f"sc{st}_{h}")
                nc.tensor.matmul(
                    sc[:, 256:512],
                    kT[t][poff:poff + 64, (jb + 1) * 128:(jb + 2) * 128],
                    qT[t][poff:poff + 64, q0:q0 + 256],
                    start=True, stop=False, skip_group_check=True,
                )
                nc.tensor.matmul(
                    sc[:, 0:128],
                    kT[t][poff:poff + 64, jb * 128:(jb + 1) * 128],
                    qT[t][poff:poff + 64, q0:q0 + 128],
                    start=True, stop=False, skip_group_check=True,
                )
                nc.tensor.matmul(
                    sc[:, 128:256],
                    kT[t][poff:poff + 64, (jb + 2) * 128:(jb + 3) * 128],
                    qT[t][poff:poff + 64, q0 + 128:q0 + 256],
                    start=True, stop=True, skip_group_check=True,
                )
                p = sb.tile([128, 512], bf16, tag="pp", bufs=8, name=f"p{st}_{h}")
                nc.scalar.activation(p[:], sc[:], AF.Exp, scale=0.125)
                nc.vector.tensor_mul(p[:], p[:], msk[st])
                pend[(st, h)] = p

            def emit_av_pair(st, j, scat_b, pair_in_b, cast_eng):
                jb = st * 2
                t = j
                p0, p1 = pend.pop((st, 2 * j)), pend.pop((st, 2 * j + 1))
                av = avps.tile([65, 512], f32, tag="av", bufs=2, name=f"av{st}_{j}")
                for half, p in ((0, p0), (1, p1)):
                    c0 = half * 256
                    h = 2 * j + half
                    nc.tensor.matmul(
                        av[:, c0:c0 + 256], vt[jb + 1][:, h * 65:h * 65 + 65],
                        p[:, 256:512],
                        start=True, stop=False, skip_group_check=True,
                    )
                    nc.tensor.matmul(
                        av[:, c0:c0 + 128], vt[jb][:, h * 65:h * 65 + 65],
                        p[:, 0:128],
                        start=False, stop=False, skip_group_check=True,
                    )
                    nc.tensor.matmul(
                        av[:, c0 + 128:c0 + 256], vt[jb + 2][:, h * 65:h * 65 + 65],
                        p[:, 128:256],
                        start=False, stop=True, skip_group_check=True,
                    )
                nc.scalar.copy(
                    scat_b[0:1, pair_in_b * 512:(pair_in_b + 1) * 512], av[64:65, :]
                )
                if cast_eng == "s":
                    nc.scalar.copy(att[st][t][0:64, :], av[0:64, 0:256])
                    nc.scalar.copy(att[st][t][64:128, :], av[0:64, 256:512])
                else:
                    nc.vector.tensor_copy(att[st][t][0:64, :], av[0:64, 0:256])
                    nc.vector.tensor_copy(att[st][t][64:128, :], av[0:64, 256:512])

            def den_batch(st, pairs, scat_b):
                n = len(pairs)
                s_b = sb.tile([2 * n, 256], f32, tag="s_b", bufs=2, name=f"s{st}_{pairs[0]}")
                nc.gpsimd.dma_start(out=s_b[:], in_=scat_b[0:1, :])
                r_b = sb.tile([2 * n, 256], f32, tag="r_b", bufs=2, name=f"r{st}_{pairs[0]}")
                nc.vector.reciprocal_approx_fast(out=r_b[:], in_=s_b[:])
                rc_b = sb.tile([1, n * 512], bf16, tag="rc_b", bufs=2, name=f"rc{st}_{pairs[0]}")
                nc.gpsimd.dma_start(out=rc_b[0:1, :], in_=r_b[:])
                for i in range(2 * n):
                    h = 2 * pairs[0] + i
                    t, poff = h // 2, (h % 2) * 64
                    rb = sb.tile([128, 256], bf16, tag="rb", bufs=4, name=f"rb{st}_{h}")
                    nc.gpsimd.partition_broadcast(rb[:], rc_b[0:1, i * 256:(i + 1) * 256])
                    asl = att[st][t][poff:poff + 64, :]
                    nc.vector.tensor_mul(asl, asl, rb[poff:poff + 64, :])

            def new_scat(st, b, npairs):
                return sb.tile([1, npairs * 512], f32, tag=f"scat{st}_{b}",
                               name=f"scat{st}_{b}")

            # ---- S1: q cb0 ----
            for m in range(4):
                for th in q_group(0, m):
                    th()
                # gate late weight DMAs to S1 progress
                if m == 0:
                    gated_dma(wq4, 4 * 4096, qT[0][0:1, 0:8])
                elif m == 1:
                    gated_dma(wq5, 5 * 4096, qT[1][0:1, 0:8])
                elif m == 2:
                    gated_dma(wq1, 1 * 4096, qT[2][0:1, 0:8])
                elif m == 3:
                    gated_dma(wq3, 3 * 4096, qT[3][0:1, 0:8])
            # ---- S2: k cb2 ----
            for gi, (m, n) in enumerate([(m, n) for m in range(4) for n in range(2)]):
                for th in k_group(2, m, n):
                    th()
                if gi == 1:
                    gated_dma(woA, 6 * 4096, kT[0][0:1, 0:8])
                elif gi == 3:
                    gated_dma(woB, 7 * 4096, kT[0][0:1, 8:16])
            # ones columns for v tiles (vector, after cst)
            for t in range(5):
                v_ones = vt[t].rearrange("p (h c) -> p h c", c=65)[:, :, 64]
                nc.vector.tensor_copy(v_ones, ones_sb[:])
            # ---- S3: v half0 t0-2 ----
            for t in range(3):
                for th in v_group(t, 0):
                    th()

            # ---- S4/S5: st0 attention interleaved with remaining proj ----
            # order matters: st0 pair j consumes kT[4+m]/qT[4+m] at slot 8+2m
            # and v-half1 of vt0-2 from pair 4 on; writers must be EMITTED
            # before their readers (tile deps snapshot at emission).
            fills = []
            for m in range(4):
                for n in range(2):
                    fills.extend(k_group(3, m, n))
                if m < 3:
                    fills.extend(v_group(m, 1))
                fills.extend(q_group(1, m))
            for t in range(3, 5):
                fills.extend(v_group(t, 0))
                fills.extend(v_group(t, 1))

            fi = [0]

            def drain(k, fills=fills, fi=fi):
                while k > 0 and fi[0] < len(fills):
                    fills[fi[0]]()
                    fi[0] += 1
                    k -= 1

            scat0_b0 = new_scat(0, 0, 4)
            scat0_b1 = new_scat(0, 1, 4)
            for j in range(8):
                emit_qk(0, 2 * j)
                drain(10)
                emit_qk(0, 2 * j + 1)
                drain(10)
                if j >= 1:
                    jj = j - 1
                    emit_av_pair(0, jj, scat0_b0 if jj < 4 else scat0_b1, jj % 4, "s")
                    if jj == 3:
                        den_batch(0, [0, 1, 2, 3], scat0_b0)
            drain(10 ** 9)
            emit_av_pair(0, 7, scat0_b1, 3, "s")
            den_batch(0, [4, 5, 6, 7], scat0_b1)

            # ---- S6: st1 attention interleaved with po2(st0) ----
            fills2 = []
            for g in range(4):
                fills2.extend(po2_group(0, g))
            fi2 = [0]

            def drain2(k, fi2=fi2):
                while k > 0 and fi2[0] < len(fills2):
                    fills2[fi2[0]]()
                    fi2[0] += 1
                    k -= 1

            scat1_b = [new_scat(1, 0, 4), new_scat(1, 1, 2),
                       new_scat(1, 2, 1), new_scat(1, 3, 1)]

            def st1_scat(j):
                if j < 4:
                    return scat1_b[0], j
                if j < 6:
                    return scat1_b[1], j - 4
                return scat1_b[j - 4], 0

            for j in range(8):
                emit_qk(1, 2 * j)
                if j >= 2:
                    drain2(5)
                emit_qk(1, 2 * j + 1)
                if j >= 2:
                    drain2(5)
                if j >= 1:
                    jj = j - 1
                    sc_b, pib = st1_scat(jj)
                    emit_av_pair(1, jj, sc_b, pib, "v")
                    if jj == 3:
                        den_batch(1, [0, 1, 2, 3], scat1_b[0])
                    elif jj == 5:
                        den_batch(1, [4, 5], scat1_b[1])
                    elif jj == 6:
                        den_batch(1, [6], scat1_b[2])
            drain2(10 ** 9)
            sc_b, pib = st1_scat(7)
            emit_av_pair(1, 7, sc_b, pib, "v")
            den_batch(1, [7], scat1_b[3])
            nc.sync.dma_start(out=out_d[:, 0:2048], in_=ot[0][:])

            # ---- S7: po2(st1) + output DMAs ----
            for g in range(4):
                for th in po2_group(1, g):
                    th()
                if g == 1:
                    nc.sync.dma_start(out=out_d[:, 2048:3072], in_=ot[1][:, 0:1024])
            nc.sync.dma_start(out=out_d[:, 3072:4096], in_=ot[1][:, 1024:2048])

    nc.finalize()
    return nc


def _get_nc():
    if "nc" not in _cached:
        _cached["nc"] = _build()
    return _cached["nc"]


def _core_inputs(x, w_qkv, w_out):
    # shared, host-prebaked weight layouts (bf16, exact SBUF layout).
    # cb0 is m-major (4 contiguous 1024-col chunks); cb1..5 are k-major.
    Wq = w_qkv.reshape(8, 128, 6, 512)
    blocks = [Wq[:, :, 0, :].reshape(8, 128, 4, 128).transpose(1, 2, 0, 3).reshape(128, 4096)]
    for cb in range(1, 6):
        blocks.append(Wq[:, :, cb, :].transpose(1, 0, 2).reshape(128, 4096))
    wq_h = np.ascontiguousarray(np.concatenate(blocks, axis=1)).astype(BF16)
    wo_h = np.ascontiguousarray(
        w_out.reshape(8, 128, 1024).transpose(1, 0, 2).reshape(128, 8192)
    ).astype(BF16)

    in_maps = []
    for c in range(NCORES):
        b, qs = c // CPB, (c % CPB) * SLOC
        xs = np.zeros((TLOC, D), dtype=np.float32)
        lo = max(0, qs - HALO)
        xs[HALO - (qs - lo):] = x[b, lo:qs + SLOC]
        xt_h = np.ascontiguousarray(
            xs.T.reshape(8, 128, TLOC).transpose(1, 0, 2).reshape(128, 8 * TLOC)
        ).astype(BF16)

        # binary {0,1} masks multiplying exp'd scores.
        i = np.arange(256)[None, None, None, :]
        j = np.arange(128)[None, None, :, None]
        st = np.arange(NST)[:, None, None, None]
        r = np.arange(3)[None, :, None, None]
        qg = qs + st * 256 + i
        kg = qs + st * 256 - HALO + r * 128 + j
        allowed = (kg <= qg) & (kg > qg - WINDOW) & (kg >= 0)
        m3 = allowed.astype(np.float32)
        mask = np.empty((NST, 128, 512), dtype=np.float32)
        mask[:, :, 0:128] = m3[:, 0, :, 0:128]
        mask[:, :, 128:256] = m3[:, 2, :, 128:256]
        mask[:, :, 256:512] = m3[:, 1]
        cst_h = np.empty((128, NST * 512 + 16), dtype=BF16)
        cst_h[:, 0:512] = mask[0]
        cst_h[:, 512:1024] = mask[1]
        cst_h[:, 1024:1040] = 1.0

        in_maps.append({"xt": xt_h, "wq": wq_h, "wo": wo_h, "cst": cst_h})
    return in_maps


def kernel(x, w_qkv, w_out, _trace=False, _trace_kwargs=None):
    from concourse.bass_utils import run_bass_kernel_spmd

    x = np.asarray(x, dtype=np.float32)
    w_qkv = np.asarray(w_qkv, dtype=np.float32)
    w_out = np.asarray(w_out, dtype=np.float32)
    nc = _get_nc()
    in_maps = _core_inputs(x, w_qkv, w_out)
    res = run_bass_kernel_spmd(
        nc, in_maps, list(range(NCORES)), trace=_trace, **(_trace_kwargs or {})
    )
    out = np.empty((B, S, D), dtype=np.float32)
    for c in range(NCORES):
        b, qs = c // CPB, (c % CPB) * SLOC
        o = np.asarray(res.results[c]["out"], dtype=np.float32)
        out[b, qs:qs + SLOC] = (
            o.reshape(128, NST, 8, 256).transpose(1, 3, 2, 0).reshape(SLOC, D)
        )
    if _trace:
        return out, res
    return out
